# revision 51
# baseline (speedup 1.0000x reference)
"""Trainium2 Bass kernel for nn_MoEConnectionProcessor.

Data-parallel over cells: 8 cores x 2560 padded cells (19683 real).

v2 layout strategy (transposed messages):
  - nbr shipped twice from host: natT [d, edge] (moving operand for all
    per-edge projections; Wm2 stays stationary across long streams) and
    nat [edge, d] subtile-major (stationary for the masked l/d
    aggregation matmuls).
  - messages live transposed [dout, edge]: proj = Wm2^T @ natT chunk,
    plus one accumulate matmul whose stationary is [cpm_nat | ones] and
    whose moving operand is a host-built masked staircase SELC carrying
    m_f selectors (rows 0-31) and a -32768 penalty row, so relu both
    applies the cur-projection broadcast and zeroes masked edges.
  - functional aggregation = DVE segmented reduce over 26-edge groups.
  - l/d aggregation masks ship pre-scaled by 1/count (bf16 hi+lo), so
    PSUM holds final normalized aggregates and evacuation is a copy.
  - gating uses reciprocal_approx_fast + gpsimd partition broadcasts.
"""

import numpy as np
import ml_dtypes
from contextlib import ExitStack

N_CELLS, K, D, HG = 19683, 26, 128, 64
NCORES = 8
NS = 2560                 # padded cells per core
E = NS * K                # 66560 edges per core
SBC = 64                  # cells per superblock (l/d agg granularity)
NSB = NS // SBC           # 40 superblocks
NSUB = 13                 # subtiles (128 edges) per superblock
HCELL = 32                # cells per half-superblock (msg granularity)
EPH = HCELL * K           # 832 edges per half-superblock
NHB = NS // HCELL         # 80 half-superblocks
NSUBT = NS * K // 128     # 520 subtiles per core
CHUNK = 512
NCHUNK = NS // CHUNK      # 5
CNF_STEPS, DTC = 3, 0.1
PEN = -32768.0

bf16 = ml_dtypes.bfloat16


def _cb_loc():
    # first local cell of subtile chi within its superblock
    return [(chi * 128) // K for chi in range(NSUB)]


CB_LOC = _cb_loc()


def _consts():
    c = {}
    ident = np.eye(128, dtype=np.float32)
    c["IDENT"] = ident.astype(bf16)                     # [128, 128]
    c["ONES2"] = np.ones((2, 128), np.float32).astype(bf16)
    c["ONES3"] = np.ones((3, 1), np.float32)
    return c


CONSTS = _consts()


def _enable_ldw_opt():
    # compile_bir_kernel hardcodes --enable-ldw-opt=false; rewrite it so
    # walrus schedules LDWEIGHTS into the PE background weight buffer.
    from concourse import bass_utils as bu
    if getattr(bu, "_ldw_patched", False):
        return
    orig = bu.run_command

    def run_command(cmd, *a, **k):
        # walrus --enable-ldw-opt=true rejects bacc's pre-split standalone
        # InstLdweights, so the flag must stay false; keep the hook for
        # future command rewrites.
        return orig(cmd, *a, **k)

    bu.run_command = run_command
    bu._ldw_patched = True
    try:
        from concourse import bass2jax as b2j
        if getattr(b2j, "run_command", None) is orig:
            b2j.run_command = run_command
    except Exception:
        pass


def _build_bass():
    import concourse.bass as bass
    import concourse.tile as tile
    from concourse import bacc, mybir

    _enable_ldw_opt()

    f32, bft, i32 = mybir.dt.float32, mybir.dt.bfloat16, mybir.dt.int32
    f8e5 = mybir.dt.float8e5
    AF = mybir.ActivationFunctionType
    OP = mybir.AluOpType
    AX = mybir.AxisListType

    nc = bacc.Bacc("TRN2", target_bir_lowering=False, debug=False,
                   num_devices=NCORES)

    def din(name, shape, dt):
        return nc.dram_tensor(name, shape, dt, kind="ExternalInput").ap()

    natT_d = din("natT", [128, E], mybir.dt.float8e4)
    nat_d = din("nat", [128, NSUBT * D], mybir.dt.float8e4)
    selc_d = din("SELC", [34, E], f8e5)
    bmsg_d = din("BMSGROW", [1, NHB * 128], bft)
    bhi_d = din("B_hi", [128, NSUBT * 12], bft)
    invf_d = din("INVF2", [2, NS], bft)
    curT_f = din("curT_f", [D, NS], f32)
    curT_b = din("curT_b", [D, NS], bft)
    wnames = ["Wl1", "Wl2", "Wm1", "Wm2", "Wu1", "Wu2", "Wc1", "Wc2"]
    WPACK_d = din("WPACK", [128, 1347], bft)
    CPACK_d = din("CPACK", [128, 7], f32)
    outT = nc.dram_tensor("outT", [D, NS], bft, kind="ExternalOutput").ap()

    with tile.TileContext(nc) as tc, ExitStack() as ctx:
        const = ctx.enter_context(tc.tile_pool(name="const", bufs=1))
        big = ctx.enter_context(tc.tile_pool(name="big", bufs=1))
        st_natT = ctx.enter_context(tc.tile_pool(name="st_natT", bufs=3))
        st_selc = ctx.enter_context(tc.tile_pool(name="st_selc", bufs=3))
        st_nat = ctx.enter_context(tc.tile_pool(name="st_nat", bufs=3))
        st_msgs = ctx.enter_context(tc.tile_pool(name="st_msgs", bufs=2))
        temp1 = ctx.enter_context(tc.tile_pool(name="temp1", bufs=2))
        psM = ctx.enter_context(tc.tile_pool(name="psM", bufs=2,
                                             space="PSUM"))
        psG = ctx.enter_context(tc.tile_pool(name="psG", bufs=2,
                                             space="PSUM"))
        psC = ctx.enter_context(tc.tile_pool(name="psC", bufs=2,
                                             space="PSUM"))

        # ---------- load constants / weights (packed: 2 DMAs) ----------
        wpack = const.tile([128, 1347], bft)
        nc.sync.dma_start(wpack[:], WPACK_d[:])
        cpack = const.tile([128, 7], f32)
        nc.sync.dma_start(cpack[:], CPACK_d[:])
        wt = {k: wpack[:, i * 128:(i + 1) * 128]
              for i, k in enumerate(wnames)}
        wg1 = wpack[:, 1024:1088]
        wg2 = wpack[0:HG, 1088:1091]
        ident = wpack[:, 1091:1219]
        ones2 = wpack[0:2, 1219:1347]
        bias = {k: cpack[0:p, i:i + 1] for i, (k, p) in enumerate(
            [("b_local", D), ("b_upd", D), ("b_cnf", D), ("b_msg", D),
             ("b_g1", HG), ("b_g2", 3)])}
        ones3 = cpack[0:3, 6:7]
        curTb = const.tile([D, NS], bft)
        nc.sync.dma_start(curTb[:], curT_b[:])
        curTf = const.tile([D, NS], f32)
        nc.sync.dma_start(curTf[:], curT_f[:])
        bhi = const.tile([128, NSUBT * 12], bft)
        nc.sync.dma_start(bhi[:], bhi_d[:])
        invf2 = const.tile([2, NS], bft)
        nc.sync.dma_start(invf2[:], invf_d[:])

        # cpm_natA [34, 80*128]: rows 0-31 cell-major cpm per half-sb
        # (computed directly: stationary = curTb cell slice, moving = Wm1),
        # row 32 = ones (penalty row), row 33 = b_msg (paired with the m_f
        # row of SELC, so the bias only lands on unmasked edges)
        cpm_natA = big.tile([34, NHB * 128], bft)
        nc.vector.memset(cpm_natA[32:33, :], 1.0)
        nc.sync.dma_start(cpm_natA[33:34, :], bmsg_d[:])
        for h2 in range(0, NHB, 4):
            pt = psC.tile([32, 512], f32, tag="p")
            for i in range(4):
                h = h2 + i
                nc.tensor.matmul(pt[:, i * 128:(i + 1) * 128],
                                 curTb[:, h * 32:(h + 1) * 32], wt["Wm1"][:],
                                 start=(i == 0), stop=(i == 3))
            nc.scalar.copy(
                cpm_natA[0:32, h2 * 128:(h2 + 4) * 128], pt[:])

        # ---------- main loop: half-superblocks ----------
        aggF32 = big.tile([128, NS], f32)     # functional agg (unnormalized)
        aggldT = big.tile([128, NSB * 128], bft)  # col t*128 + 2c+m (l,d)

        def do_half(h):
            natT_h = st_natT.tile([128, EPH], mybir.dt.float8e4, tag="natT")
            nc.sync.dma_start(natT_h[:], natT_d[:, h * EPH:(h + 1) * EPH])
            selc_h = st_selc.tile([34, EPH], f8e5, tag="selc")
            nc.sync.dma_start(selc_h[:], selc_d[:, h * EPH:(h + 1) * EPH])

            # [128, 1024] so the slot is exactly 2 PSUM banks (bank-aligned)
            pmsg = psM.tile([128, 1024], f32, tag="pm")
            mm = nc.tensor.matmul(pmsg[:, 0:512], wt["Wm2"][:],
                                  natT_h[:, 0:512], start=True, stop=False)
            mm2 = nc.tensor.matmul(pmsg[:, 512:EPH], wt["Wm2"][:],
                                   natT_h[:, 512:EPH], start=True, stop=False)
            mm2.ins.ldweights = False
            stat = cpm_natA[:, h * 128:(h + 1) * 128]
            mm3 = nc.tensor.matmul(pmsg[:, 0:512], stat, selc_h[:, 0:512],
                                   start=False, stop=True)
            mm4 = nc.tensor.matmul(pmsg[:, 512:EPH], stat,
                                   selc_h[:, 512:EPH], start=False, stop=True)
            mm4.ins.ldweights = False

            msgs = st_msgs.tile([128, EPH], bft, tag="msgs")
            nc.scalar.activation(msgs[:], pmsg[:, 0:EPH], AF.Relu)
            # pairwise pre-add on gpsimd halves the DVE reduce volume
            mv = msgs[:].rearrange("p (c k) -> p c k", k=K)
            msum = st_msgs.tile([128, HCELL, 13], f32, tag="msum")
            nc.gpsimd.tensor_tensor(msum[:], mv[:, :, 0:13], mv[:, :, 13:26],
                                    OP.add)
            nc.vector.tensor_reduce(
                aggF32[:, h * HCELL:(h + 1) * HCELL], msum[:],
                AX.X, OP.add)

        def do_ld(t):
            nat_t = st_nat.tile([128, NSUB * 128], mybir.dt.float8e4, tag="nat")
            nc.sync.dma_start(
                nat_t[:], nat_d[:, t * NSUB * 128:(t + 1) * NSUB * 128])
            # full-bank slot ([128, 512] f32 = 1 bank); only 128 cols used
            pagg_t = psG.tile([128, 512], f32, tag="pg")
            pagg = pagg_t[:, 0:128]
            for s in range(NSUB):
                sg = t * NSUB + s
                cb2 = 2 * CB_LOC[s]
                w = min(6, SBC - CB_LOC[s])
                nat_s = nat_t[:, s * 128:(s + 1) * 128]
                nc.tensor.matmul(pagg[:, cb2:cb2 + 2 * w], nat_s,
                                 bhi[:, sg * 12:sg * 12 + 2 * w],
                                 start=(s == 0), stop=(s == NSUB - 1))
            nc.vector.tensor_copy(aggldT[:, t * 128:(t + 1) * 128], pagg[:])

        # second-stage tiles (written per chunk in the tail)
        aggFb = big.tile([128, NS], bft)
        localT = big.tile([128, NS], bft)
        funcT = big.tile([128, NS], bft)
        s_steps = [big.tile([128, NS], f32, tag=f"s{i % 2}", name=f"s{i}")
                   for i in range(2)]
        sbf_steps = [big.tile([128, NS], bft, tag=f"sbf{i}", name=f"sb{i}")
                     for i in range(2)]

        def agg_view(base_off, ch):
            # aggldT cols (t*128 + 2c + m) for cells of chunk ch
            v = aggldT[:, ch * 8 * 128 + base_off:(ch + 1) * 8 * 128:2]
            return v.rearrange("p (t c) -> p t c", c=64)

        def norm_local_func(ch):
            sl = slice(ch * CHUNK, (ch + 1) * CHUNK)
            # normalize functional aggregate for this chunk
            pb = psC.tile([128, CHUNK], f32, tag="p")
            nc.tensor.matmul(pb[:], ones2[:], invf2[:, sl], start=True,
                             stop=True)
            nc.vector.tensor_tensor(aggFb[:, sl], aggF32[:, sl], pb[:],
                                    OP.mult)
            # local expert
            pl = psC.tile([128, CHUNK], f32, tag="p")
            nc.tensor.matmul(pl[:], wt["Wl1"][:], curTb[:, sl], start=True,
                             stop=False)
            nc.tensor.matmul(
                pl[:].rearrange("p (t c) -> p t c", c=64),
                wt["Wl2"][:], agg_view(0, ch), start=False, stop=True)
            nc.scalar.activation(localT[:, sl], pl[:], AF.Tanh,
                                 bias=bias["b_local"][:])
            # functional expert
            pf = psC.tile([128, CHUNK], f32, tag="p")
            nc.tensor.matmul(pf[:], wt["Wu1"][:], curTb[:, sl], start=True,
                             stop=False)
            nc.tensor.matmul(pf[:], wt["Wu2"][:], aggFb[:, sl],
                             start=False, stop=True)
            nc.scalar.activation(funcT[:, sl], pf[:], AF.Tanh,
                                 bias=bias["b_upd"][:])

        def cnf_step(step, ch, s_prev, s_prev_bf):
            sl = slice(ch * CHUNK, (ch + 1) * CHUNK)
            s_next = s_steps[step % 2]
            pp = psC.tile([128, CHUNK], f32, tag="p")
            nc.tensor.matmul(pp[:], wt["Wc1"][:], s_prev_bf[:, sl],
                             start=True, stop=False)
            nc.tensor.matmul(
                pp[:].rearrange("p (t c) -> p t c", c=64),
                wt["Wc2"][:], agg_view(1, ch), start=False, stop=True)
            th = temp1.tile([128, CHUNK], f32, tag="th")
            nc.scalar.activation(th[:], pp[:], AF.Tanh,
                                 bias=bias["b_cnf"][:])
            nc.vector.scalar_tensor_tensor(
                s_next[:, sl], th[:], DTC, s_prev[:, sl],
                OP.mult, OP.add)
            if step < CNF_STEPS - 1:
                nb = sbf_steps[step]
                nc.vector.tensor_copy(nb[:, sl], s_next[:, sl])

        def gate_mix(ch, s_prev):
            sl = slice(ch * CHUNK, (ch + 1) * CHUNK)
            ph = psC.tile([HG, CHUNK], f32, tag="p")
            nc.tensor.matmul(ph[:], wg1[:], curTb[:, sl], start=True,
                             stop=True)
            hT = temp1.tile([HG, CHUNK], bft, tag="hT")
            nc.scalar.activation(hT[:], ph[:], AF.Relu, bias=bias["b_g1"][:])
            pz = psC.tile([3, CHUNK], f32, tag="p")
            nc.tensor.matmul(pz[:], wg2[:], hT[:], start=True, stop=True)
            e3 = temp1.tile([3, CHUNK], f32, tag="e3")
            nc.scalar.activation(e3[:], pz[:], AF.Exp, bias=bias["b_g2"][:])
            p1_t = psG.tile([128, 512], f32, tag="pg")
            p1 = p1_t[0:1, 0:CHUNK]
            nc.tensor.matmul(p1[:], ones3[:], e3[:], start=True, stop=True)
            rec = temp1.tile([1, CHUNK], f32, tag="rec")
            nc.vector.reciprocal_approx_fast(rec[:], p1[:])
            rbc = temp1.tile([128, CHUNK], f32, tag="rbc")
            nc.gpsimd.partition_broadcast(rbc[:], rec[:])
            ge = []
            for m in range(3):
                # partition_broadcast needs its input on partition 0; DMA the
                # gate row down from partition m first
                erow = temp1.tile([1, CHUNK], f32, tag=f"erow{m}",
                                  name=f"erow{m}")
                nc.sync.dma_start(erow[:], e3[m:m + 1, :])
                g = temp1.tile([128, CHUNK], f32, tag=f"ge{m}",
                               name=f"ge{m}")
                nc.gpsimd.partition_broadcast(g[:], erow[:])
                ge.append(g)
            acc = temp1.tile([128, CHUNK], f32, tag="acc")
            tmp = temp1.tile([128, CHUNK], f32, tag="tmp")
            accb = temp1.tile([128, CHUNK], bft, tag="accb")
            nc.vector.tensor_tensor(acc[:], localT[:, sl], ge[0][:], OP.mult)
            nc.vector.tensor_tensor(tmp[:], funcT[:, sl], ge[1][:], OP.mult)
            nc.vector.tensor_tensor(acc[:], acc[:], tmp[:], OP.add)
            nc.vector.tensor_tensor(tmp[:], s_prev[:, sl], ge[2][:], OP.mult)
            nc.vector.tensor_tensor(acc[:], acc[:], tmp[:], OP.add)
            nc.vector.tensor_tensor(accb[:], acc[:], rbc[:], OP.mult)
            nc.sync.dma_start(outT[:, sl], accb[:])

        for t in range(NSB):
            do_ld(t)
            do_half(2 * t)
            do_half(2 * t + 1)
        for ch in range(NCHUNK):
            norm_local_func(ch)
        # CNF step-outer so chunks pipeline within each step
        prevs = [(curTf, curTb)] * NCHUNK
        for step in range(CNF_STEPS):
            for ch in range(NCHUNK):
                s_prev, s_prev_bf = prevs[ch]
                cnf_step(step, ch, s_prev, s_prev_bf)
                prevs[ch] = (s_steps[step % 2],
                             sbf_steps[step] if step < CNF_STEPS - 1
                             else None)
        for ch in range(NCHUNK):
            gate_mix(ch, s_steps[(CNF_STEPS - 1) % 2])

    nc.compile()
    return nc


_NC_CACHE = None


def _get_nc():
    global _NC_CACHE
    if _NC_CACHE is None:
        _NC_CACHE = _build_bass()
    return _NC_CACHE


def _split_hilo(w):
    hi = w.astype(bf16)
    lo = (w - hi.astype(np.float32)).astype(bf16)
    return hi, lo


def _prep_core_inputs(cur, nbr, conn, weights):
    """cur [NS, D] f32, nbr [NS, K, D] f32, conn [NS, K] i32 -> input map."""
    m = {}
    f8n = ml_dtypes.float8_e4m3fn
    x = nbr.reshape(E, D).astype(f8n)
    m["natT"] = np.ascontiguousarray(nbr.reshape(E, D).T.astype(f8n))
    m["nat"] = np.ascontiguousarray(
        x.reshape(NSUBT, 128, D).transpose(1, 0, 2)).reshape(128, NSUBT * D)

    cf = conn.reshape(E)
    mf = (cf == 1).astype(np.float32)
    ml = (cf == 0).astype(np.float32)
    md = (cf == 2).astype(np.float32)

    # SELC: masked staircase + penalty + m_f rows (fp8e5: all values exact)
    f8 = ml_dtypes.float8_e5m2
    selc = np.zeros((34, E), f8)
    eidx = np.arange(E)
    cl32 = (eidx // K) % HCELL
    selc[cl32, eidx] = mf.astype(f8)
    selc[32, :] = (PEN * (1.0 - mf)).astype(f8)
    selc[33, :] = mf.astype(f8)
    m["SELC"] = selc

    # per-cell inverse counts
    cnt_l = ml.reshape(NS, K).sum(1)
    cnt_f = mf.reshape(NS, K).sum(1)
    cnt_d = md.reshape(NS, K).sum(1)
    inv_l = 1.0 / np.maximum(cnt_l, 1.0)
    inv_f = 1.0 / np.maximum(cnt_f, 1.0)
    inv_d = 1.0 / np.maximum(cnt_d, 1.0)

    # B_hi/B_lo: staircase * mask * inv, interleaved (l,d) per cell
    cell = eidx // K          # global cell per edge
    cl64 = cell % SBC         # local cell within superblock
    s_of_e = eidx // 128      # subtile
    j = cl64 - np.asarray(CB_LOC)[s_of_e % NSUB]   # 0..5
    p_of_e = eidx % 128
    w_l = ml * inv_l[cell]
    w_d = md * inv_d[cell]
    B = np.zeros((128, NSUBT * 12), np.float32)
    B[p_of_e, s_of_e * 12 + 2 * j] = w_l
    B[p_of_e, s_of_e * 12 + 2 * j + 1] = w_d
    m["B_hi"] = B.astype(bf16)

    ihi, ilo = _split_hilo(inv_f.astype(np.float32))
    m["INVF2"] = np.stack([ihi, ilo], axis=0)                   # [2, NS]

    ct = np.ascontiguousarray(cur.T)
    m["curT_f"] = ct.astype(np.float32)
    m["curT_b"] = ct.astype(bf16)

    Wl, Wm, Wu, Wc = (weights["W_local"], weights["W_msg"],
                      weights["W_upd"], weights["W_cnf"])
    wpack = np.zeros((128, 1347), np.float32)
    for i, wmat in enumerate([Wl[:D], Wl[D:], Wm[:D], Wm[D:],
                              Wu[:D], Wu[D:], Wc[:D], Wc[D:]]):
        wpack[:, i * 128:(i + 1) * 128] = wmat
    wpack[:, 1024:1088] = weights["W_g1"]
    wpack[0:HG, 1088:1091] = weights["W_g2"]
    wpack[:, 1091:1219] = np.eye(128, dtype=np.float32)
    wpack[0:2, 1219:1347] = 1.0
    m["WPACK"] = wpack.astype(bf16)
    cpack = np.zeros((128, 7), np.float32)
    cpack[0:D, 0] = weights["b_local"]
    cpack[0:D, 1] = weights["b_upd"]
    cpack[0:D, 2] = weights["b_cnf"]
    cpack[0:D, 3] = weights["b_msg"]
    cpack[0:HG, 4] = weights["b_g1"]
    cpack[0:3, 5] = weights["b_g2"]
    cpack[0:3, 6] = 1.0
    m["CPACK"] = cpack
    m["BMSGROW"] = np.tile(weights["b_msg"].astype(np.float32),
                           NHB).reshape(1, NHB * 128).astype(bf16)
    return m


def kernel(**inputs):
    from concourse.bass_utils import run_bass_kernel_spmd

    cur = np.asarray(inputs["current_state"], np.float32)
    nbr = np.asarray(inputs["neighbor_states"], np.float32)
    conn = np.asarray(inputs["conn_type"], np.int32)
    weights = {k: np.asarray(v, np.float32) for k, v in inputs.items()
               if k not in ("current_state", "neighbor_states", "conn_type")}

    npad = NCORES * NS
    cur_p = np.zeros((npad, D), np.float32)
    cur_p[:N_CELLS] = cur
    nbr_p = np.zeros((npad, K, D), np.float32)
    nbr_p[:N_CELLS] = nbr
    conn_p = np.full((npad, K), 3, np.int32)
    conn_p[:N_CELLS] = conn

    in_maps = []
    for c in range(NCORES):
        sl = slice(c * NS, (c + 1) * NS)
        in_maps.append(_prep_core_inputs(cur_p[sl], nbr_p[sl], conn_p[sl],
                                         weights))
    nc = _get_nc()
    res = run_bass_kernel_spmd(nc, in_maps, list(range(NCORES)))
    out = np.concatenate([res.results[c]["outT"].T for c in range(NCORES)],
                         axis=0)
    return np.ascontiguousarray(out[:N_CELLS]).astype(np.float32)


if __name__ == "__main__":
    pass


# revision 53
# speedup vs baseline: 1.0104x; 1.0104x over previous
"""Trainium2 Bass kernel for nn_MoEConnectionProcessor.

Data-parallel over cells: 8 cores x 2560 padded cells (19683 real).

v2 layout strategy (transposed messages):
  - nbr shipped twice from host: natT [d, edge] (moving operand for all
    per-edge projections; Wm2 stays stationary across long streams) and
    nat [edge, d] subtile-major (stationary for the masked l/d
    aggregation matmuls).
  - messages live transposed [dout, edge]: proj = Wm2^T @ natT chunk,
    plus one accumulate matmul whose stationary is [cpm_nat | ones] and
    whose moving operand is a host-built masked staircase SELC carrying
    m_f selectors (rows 0-31) and a -32768 penalty row, so relu both
    applies the cur-projection broadcast and zeroes masked edges.
  - functional aggregation = DVE segmented reduce over 26-edge groups.
  - l/d aggregation masks ship pre-scaled by 1/count (bf16 hi+lo), so
    PSUM holds final normalized aggregates and evacuation is a copy.
  - gating uses reciprocal_approx_fast + gpsimd partition broadcasts.
"""

import numpy as np
import ml_dtypes
from contextlib import ExitStack

N_CELLS, K, D, HG = 19683, 26, 128, 64
NCORES = 8
NS = 2560                 # padded cells per core
E = NS * K                # 66560 edges per core
SBC = 64                  # cells per superblock (l/d agg granularity)
NSB = NS // SBC           # 40 superblocks
NSUB = 13                 # subtiles (128 edges) per superblock
HCELL = 32                # cells per half-superblock (msg granularity)
EPH = HCELL * K           # 832 edges per half-superblock
NHB = NS // HCELL         # 80 half-superblocks
NSUBT = NS * K // 128     # 520 subtiles per core
CHUNK = 512
NCHUNK = NS // CHUNK      # 5
CNF_STEPS, DTC = 3, 0.1
PEN = -32768.0

bf16 = ml_dtypes.bfloat16


def _cb_loc():
    # first local cell of subtile chi within its superblock
    return [(chi * 128) // K for chi in range(NSUB)]


CB_LOC = _cb_loc()


def _consts():
    c = {}
    ident = np.eye(128, dtype=np.float32)
    c["IDENT"] = ident.astype(bf16)                     # [128, 128]
    c["ONES2"] = np.ones((2, 128), np.float32).astype(bf16)
    c["ONES3"] = np.ones((3, 1), np.float32)
    return c


CONSTS = _consts()


def _enable_ldw_opt():
    # compile_bir_kernel hardcodes --enable-ldw-opt=false; rewrite it so
    # walrus schedules LDWEIGHTS into the PE background weight buffer.
    from concourse import bass_utils as bu
    if getattr(bu, "_ldw_patched", False):
        return
    orig = bu.run_command

    def run_command(cmd, *a, **k):
        # walrus --enable-ldw-opt=true rejects bacc's pre-split standalone
        # InstLdweights, so the flag must stay false; keep the hook for
        # future command rewrites.
        return orig(cmd, *a, **k)

    bu.run_command = run_command
    bu._ldw_patched = True
    try:
        from concourse import bass2jax as b2j
        if getattr(b2j, "run_command", None) is orig:
            b2j.run_command = run_command
    except Exception:
        pass


def _build_bass():
    import concourse.bass as bass
    import concourse.tile as tile
    from concourse import bacc, mybir

    _enable_ldw_opt()

    f32, bft, i32 = mybir.dt.float32, mybir.dt.bfloat16, mybir.dt.int32
    f8e5 = mybir.dt.float8e5
    AF = mybir.ActivationFunctionType
    OP = mybir.AluOpType
    AX = mybir.AxisListType

    nc = bacc.Bacc("TRN2", target_bir_lowering=False, debug=False,
                   num_devices=NCORES)

    def din(name, shape, dt):
        return nc.dram_tensor(name, shape, dt, kind="ExternalInput").ap()

    natT_d = din("natT", [128, E], mybir.dt.float8e4)
    nat_d = din("nat", [128, NSUBT * D], mybir.dt.float8e4)
    selc_d = din("SELC", [33, E], f8e5)
    bhi_d = din("B_hi", [128, NSUBT * 12], bft)
    invf_d = din("INVF2", [2, NS], bft)
    curT_f = din("curT_f", [D, NS], f32)
    curT_b = din("curT_b", [D, NS], bft)
    wnames = ["Wl1", "Wl2", "Wm1", "Wm2", "Wu1", "Wu2", "Wc1", "Wc2"]
    WPACK_d = din("WPACK", [128, 1347], bft)
    CPACK_d = din("CPACK", [128, 7], f32)
    outT = nc.dram_tensor("outT", [D, NS], bft, kind="ExternalOutput").ap()

    with tile.TileContext(nc) as tc, ExitStack() as ctx:
        const = ctx.enter_context(tc.tile_pool(name="const", bufs=1))
        big = ctx.enter_context(tc.tile_pool(name="big", bufs=1))
        st_natT = ctx.enter_context(tc.tile_pool(name="st_natT", bufs=3))
        st_selc = ctx.enter_context(tc.tile_pool(name="st_selc", bufs=3))
        st_nat = ctx.enter_context(tc.tile_pool(name="st_nat", bufs=3))
        st_msgs = ctx.enter_context(tc.tile_pool(name="st_msgs", bufs=2))
        temp1 = ctx.enter_context(tc.tile_pool(name="temp1", bufs=2))
        psM = ctx.enter_context(tc.tile_pool(name="psM", bufs=2,
                                             space="PSUM"))
        psG = ctx.enter_context(tc.tile_pool(name="psG", bufs=2,
                                             space="PSUM"))
        psC = ctx.enter_context(tc.tile_pool(name="psC", bufs=2,
                                             space="PSUM"))

        # ---------- load constants / weights (packed: 2 DMAs) ----------
        wpack = const.tile([128, 1347], bft)
        nc.sync.dma_start(wpack[:], WPACK_d[:])
        cpack = const.tile([128, 7], f32)
        nc.sync.dma_start(cpack[:], CPACK_d[:])
        wt = {k: wpack[:, i * 128:(i + 1) * 128]
              for i, k in enumerate(wnames)}
        wg1 = wpack[:, 1024:1088]
        wg2 = wpack[0:HG, 1088:1091]
        ident = wpack[:, 1091:1219]
        ones2 = wpack[0:2, 1219:1347]
        bias = {k: cpack[0:p, i:i + 1] for i, (k, p) in enumerate(
            [("b_local", D), ("b_upd", D), ("b_cnf", D), ("b_msg", D),
             ("b_g1", HG), ("b_g2", 3)])}
        ones3 = cpack[0:3, 6:7]
        curTb = const.tile([D, NS], bft)
        nc.sync.dma_start(curTb[:], curT_b[:])
        curTf = const.tile([D, NS], f32)
        nc.sync.dma_start(curTf[:], curT_f[:])
        bhi = const.tile([128, NSUBT * 12], bft)
        nc.sync.dma_start(bhi[:], bhi_d[:])
        invf2 = const.tile([2, NS], bft)
        nc.sync.dma_start(invf2[:], invf_d[:])

        # ---------- cpmT = Wm1.T @ curT + b_msg ----------
        cpmT = big.tile([D, NS], bft)
        for ch in range(NCHUNK):
            pm = psC.tile([128, CHUNK], f32, tag="p")
            sl = slice(ch * CHUNK, (ch + 1) * CHUNK)
            mm = nc.tensor.matmul(pm[:], wt["Wm1"][:], curTb[:, sl],
                                  start=True, stop=True)
            if ch > 0:
                mm.ins.ldweights = False
            nc.scalar.activation(cpmT[:, sl], pm[:], AF.Identity,
                                 bias=bias["b_msg"][:])

        # cpm_natA [33, 80*128]: rows 0-31 cell-major cpm per half-sb,
        # row 32 = ones (for the SELC penalty row)
        cpm_natA = big.tile([33, NHB * 128], bft)
        nc.vector.memset(cpm_natA[32:33, :], 1.0)
        for h2 in range(0, NHB, 4):
            pt = psC.tile([32, 512], bft, tag="p")
            for i in range(4):
                h = h2 + i
                nc.tensor.transpose(pt[:, i * 128:(i + 1) * 128],
                                    cpmT[:, h * 32:(h + 1) * 32], ident[:])
            nc.scalar.copy(
                cpm_natA[0:32, h2 * 128:(h2 + 4) * 128], pt[:])

        # ---------- main loop: half-superblocks ----------
        aggF32 = big.tile([128, NS], f32)     # functional agg (unnormalized)
        aggldT = big.tile([128, NSB * 128], bft)  # col t*128 + 2c+m (l,d)

        def do_half(h):
            natT_h = st_natT.tile([128, EPH], mybir.dt.float8e4, tag="natT")
            nc.sync.dma_start(natT_h[:], natT_d[:, h * EPH:(h + 1) * EPH])
            selc_h = st_selc.tile([33, EPH], f8e5, tag="selc")
            nc.sync.dma_start(selc_h[:], selc_d[:, h * EPH:(h + 1) * EPH])

            # [128, 1024] so the slot is exactly 2 PSUM banks (bank-aligned)
            pmsg = psM.tile([128, 1024], f32, tag="pm")
            mm = nc.tensor.matmul(pmsg[:, 0:512], wt["Wm2"][:],
                                  natT_h[:, 0:512], start=True, stop=False)
            mm2 = nc.tensor.matmul(pmsg[:, 512:EPH], wt["Wm2"][:],
                                   natT_h[:, 512:EPH], start=True, stop=False)
            mm2.ins.ldweights = False
            stat = cpm_natA[:, h * 128:(h + 1) * 128]
            mm3 = nc.tensor.matmul(pmsg[:, 0:512], stat, selc_h[:, 0:512],
                                   start=False, stop=True)
            mm4 = nc.tensor.matmul(pmsg[:, 512:EPH], stat,
                                   selc_h[:, 512:EPH], start=False, stop=True)
            mm4.ins.ldweights = False

            msgs = st_msgs.tile([128, EPH], bft, tag="msgs")
            nc.scalar.activation(msgs[:], pmsg[:, 0:EPH], AF.Relu)
            # pairwise pre-add on gpsimd halves the DVE reduce volume
            mv = msgs[:].rearrange("p (c k) -> p c k", k=K)
            msum = st_msgs.tile([128, HCELL, 13], f32, tag="msum")
            nc.gpsimd.tensor_tensor(msum[:], mv[:, :, 0:13], mv[:, :, 13:26],
                                    OP.add)
            nc.vector.tensor_reduce(
                aggF32[:, h * HCELL:(h + 1) * HCELL], msum[:],
                AX.X, OP.add)

        def do_ld(t):
            nat_t = st_nat.tile([128, NSUB * 128], mybir.dt.float8e4, tag="nat")
            nc.sync.dma_start(
                nat_t[:], nat_d[:, t * NSUB * 128:(t + 1) * NSUB * 128])
            # full-bank slot ([128, 512] f32 = 1 bank); only 128 cols used
            pagg_t = psG.tile([128, 512], f32, tag="pg")
            pagg = pagg_t[:, 0:128]
            for s in range(NSUB):
                sg = t * NSUB + s
                cb2 = 2 * CB_LOC[s]
                w = min(6, SBC - CB_LOC[s])
                nat_s = nat_t[:, s * 128:(s + 1) * 128]
                nc.tensor.matmul(pagg[:, cb2:cb2 + 2 * w], nat_s,
                                 bhi[:, sg * 12:sg * 12 + 2 * w],
                                 start=(s == 0), stop=(s == NSUB - 1))
            nc.vector.tensor_copy(aggldT[:, t * 128:(t + 1) * 128], pagg[:])

        # second-stage tiles (written per chunk in the tail)
        aggFb = big.tile([128, NS], bft)
        localT = big.tile([128, NS], bft)
        funcT = big.tile([128, NS], bft)
        # CNF state kept in bf16: the Euler-step matmul consumes s_next
        # directly, no re-cast copy needed
        s_steps = [big.tile([128, NS], bft, tag=f"s{i % 2}", name=f"s{i}")
                   for i in range(2)]

        def agg_view(base_off, ch):
            # aggldT cols (t*128 + 2c + m) for cells of chunk ch
            v = aggldT[:, ch * 8 * 128 + base_off:(ch + 1) * 8 * 128:2]
            return v.rearrange("p (t c) -> p t c", c=64)

        def norm_local_func(ch):
            sl = slice(ch * CHUNK, (ch + 1) * CHUNK)
            # normalize functional aggregate for this chunk
            pb = psC.tile([128, CHUNK], f32, tag="p")
            nc.tensor.matmul(pb[:], ones2[:], invf2[:, sl], start=True,
                             stop=True)
            nc.vector.tensor_tensor(aggFb[:, sl], aggF32[:, sl], pb[:],
                                    OP.mult)
            # local expert
            pl = psC.tile([128, CHUNK], f32, tag="p")
            nc.tensor.matmul(pl[:], wt["Wl1"][:], curTb[:, sl], start=True,
                             stop=False)
            nc.tensor.matmul(
                pl[:].rearrange("p (t c) -> p t c", c=64),
                wt["Wl2"][:], agg_view(0, ch), start=False, stop=True)
            nc.scalar.activation(localT[:, sl], pl[:], AF.Tanh,
                                 bias=bias["b_local"][:])
            # functional expert
            pf = psC.tile([128, CHUNK], f32, tag="p")
            nc.tensor.matmul(pf[:], wt["Wu1"][:], curTb[:, sl], start=True,
                             stop=False)
            nc.tensor.matmul(pf[:], wt["Wu2"][:], aggFb[:, sl],
                             start=False, stop=True)
            nc.scalar.activation(funcT[:, sl], pf[:], AF.Tanh,
                                 bias=bias["b_upd"][:])

        def cnf_step(step, ch, s_prev, s_prev_bf):
            sl = slice(ch * CHUNK, (ch + 1) * CHUNK)
            s_next = s_steps[step % 2]
            pp = psC.tile([128, CHUNK], f32, tag="p")
            nc.tensor.matmul(pp[:], wt["Wc1"][:], s_prev_bf[:, sl],
                             start=True, stop=False)
            nc.tensor.matmul(
                pp[:].rearrange("p (t c) -> p t c", c=64),
                wt["Wc2"][:], agg_view(1, ch), start=False, stop=True)
            th = temp1.tile([128, CHUNK], f32, tag="th")
            nc.scalar.activation(th[:], pp[:], AF.Tanh,
                                 bias=bias["b_cnf"][:])
            nc.vector.scalar_tensor_tensor(
                s_next[:, sl], th[:], DTC, s_prev[:, sl],
                OP.mult, OP.add)

        def gate_mix(ch, s_prev):
            sl = slice(ch * CHUNK, (ch + 1) * CHUNK)
            ph = psC.tile([HG, CHUNK], f32, tag="p")
            nc.tensor.matmul(ph[:], wg1[:], curTb[:, sl], start=True,
                             stop=True)
            hT = temp1.tile([HG, CHUNK], bft, tag="hT")
            nc.scalar.activation(hT[:], ph[:], AF.Relu, bias=bias["b_g1"][:])
            pz = psC.tile([3, CHUNK], f32, tag="p")
            nc.tensor.matmul(pz[:], wg2[:], hT[:], start=True, stop=True)
            e3 = temp1.tile([3, CHUNK], f32, tag="e3")
            nc.scalar.activation(e3[:], pz[:], AF.Exp, bias=bias["b_g2"][:])
            p1_t = psG.tile([128, 512], f32, tag="pg")
            p1 = p1_t[0:1, 0:CHUNK]
            nc.tensor.matmul(p1[:], ones3[:], e3[:], start=True, stop=True)
            rec = temp1.tile([1, CHUNK], f32, tag="rec")
            nc.vector.reciprocal_approx_fast(rec[:], p1[:])
            rbc = temp1.tile([128, CHUNK], f32, tag="rbc")
            nc.gpsimd.partition_broadcast(rbc[:], rec[:])
            ge = []
            for m in range(3):
                # partition_broadcast needs its input on partition 0; DMA the
                # gate row down from partition m first
                erow = temp1.tile([1, CHUNK], f32, tag=f"erow{m}",
                                  name=f"erow{m}")
                nc.sync.dma_start(erow[:], e3[m:m + 1, :])
                g = temp1.tile([128, CHUNK], f32, tag=f"ge{m}",
                               name=f"ge{m}")
                nc.gpsimd.partition_broadcast(g[:], erow[:])
                ge.append(g)
            acc = temp1.tile([128, CHUNK], f32, tag="acc")
            tmp = temp1.tile([128, CHUNK], f32, tag="tmp")
            accb = temp1.tile([128, CHUNK], bft, tag="accb")
            nc.vector.tensor_tensor(acc[:], localT[:, sl], ge[0][:], OP.mult)
            nc.vector.tensor_tensor(tmp[:], funcT[:, sl], ge[1][:], OP.mult)
            nc.vector.tensor_tensor(acc[:], acc[:], tmp[:], OP.add)
            nc.vector.tensor_tensor(tmp[:], s_prev[:, sl], ge[2][:], OP.mult)
            nc.vector.tensor_tensor(acc[:], acc[:], tmp[:], OP.add)
            nc.vector.tensor_tensor(accb[:], acc[:], rbc[:], OP.mult)
            nc.sync.dma_start(outT[:, sl], accb[:])

        for t in range(NSB):
            do_ld(t)
            do_half(2 * t)
            do_half(2 * t + 1)
        for ch in range(NCHUNK):
            norm_local_func(ch)
        # CNF step-outer so chunks pipeline within each step
        prevs = [(curTf, curTb)] * NCHUNK
        for step in range(CNF_STEPS):
            for ch in range(NCHUNK):
                s_prev, s_prev_bf = prevs[ch]
                cnf_step(step, ch, s_prev, s_prev_bf)
                prevs[ch] = (s_steps[step % 2], s_steps[step % 2])
        for ch in range(NCHUNK):
            gate_mix(ch, s_steps[(CNF_STEPS - 1) % 2])

    nc.compile()
    return nc


_NC_CACHE = None


def _get_nc():
    global _NC_CACHE
    if _NC_CACHE is None:
        _NC_CACHE = _build_bass()
    return _NC_CACHE


def _split_hilo(w):
    hi = w.astype(bf16)
    lo = (w - hi.astype(np.float32)).astype(bf16)
    return hi, lo


def _prep_core_inputs(cur, nbr, conn, weights):
    """cur [NS, D] f32, nbr [NS, K, D] f32, conn [NS, K] i32 -> input map."""
    m = {}
    f8n = ml_dtypes.float8_e4m3fn
    x = nbr.reshape(E, D).astype(f8n)
    m["natT"] = np.ascontiguousarray(nbr.reshape(E, D).T.astype(f8n))
    m["nat"] = np.ascontiguousarray(
        x.reshape(NSUBT, 128, D).transpose(1, 0, 2)).reshape(128, NSUBT * D)

    cf = conn.reshape(E)
    mf = (cf == 1).astype(np.float32)
    ml = (cf == 0).astype(np.float32)
    md = (cf == 2).astype(np.float32)

    # SELC: masked staircase + penalty row (fp8e5: 0/1/-32768 all exact)
    f8 = ml_dtypes.float8_e5m2
    selc = np.zeros((33, E), f8)
    eidx = np.arange(E)
    cl32 = (eidx // K) % HCELL
    selc[cl32, eidx] = mf.astype(f8)
    selc[32, :] = (PEN * (1.0 - mf)).astype(f8)
    m["SELC"] = selc

    # per-cell inverse counts
    cnt_l = ml.reshape(NS, K).sum(1)
    cnt_f = mf.reshape(NS, K).sum(1)
    cnt_d = md.reshape(NS, K).sum(1)
    inv_l = 1.0 / np.maximum(cnt_l, 1.0)
    inv_f = 1.0 / np.maximum(cnt_f, 1.0)
    inv_d = 1.0 / np.maximum(cnt_d, 1.0)

    # B_hi/B_lo: staircase * mask * inv, interleaved (l,d) per cell
    cell = eidx // K          # global cell per edge
    cl64 = cell % SBC         # local cell within superblock
    s_of_e = eidx // 128      # subtile
    j = cl64 - np.asarray(CB_LOC)[s_of_e % NSUB]   # 0..5
    p_of_e = eidx % 128
    w_l = ml * inv_l[cell]
    w_d = md * inv_d[cell]
    B = np.zeros((128, NSUBT * 12), np.float32)
    B[p_of_e, s_of_e * 12 + 2 * j] = w_l
    B[p_of_e, s_of_e * 12 + 2 * j + 1] = w_d
    m["B_hi"] = B.astype(bf16)

    ihi, ilo = _split_hilo(inv_f.astype(np.float32))
    m["INVF2"] = np.stack([ihi, ilo], axis=0)                   # [2, NS]

    ct = np.ascontiguousarray(cur.T)
    m["curT_f"] = ct.astype(np.float32)
    m["curT_b"] = ct.astype(bf16)

    Wl, Wm, Wu, Wc = (weights["W_local"], weights["W_msg"],
                      weights["W_upd"], weights["W_cnf"])
    wpack = np.zeros((128, 1347), np.float32)
    for i, wmat in enumerate([Wl[:D], Wl[D:], Wm[:D], Wm[D:],
                              Wu[:D], Wu[D:], Wc[:D], Wc[D:]]):
        wpack[:, i * 128:(i + 1) * 128] = wmat
    wpack[:, 1024:1088] = weights["W_g1"]
    wpack[0:HG, 1088:1091] = weights["W_g2"]
    wpack[:, 1091:1219] = np.eye(128, dtype=np.float32)
    wpack[0:2, 1219:1347] = 1.0
    m["WPACK"] = wpack.astype(bf16)
    cpack = np.zeros((128, 7), np.float32)
    cpack[0:D, 0] = weights["b_local"]
    cpack[0:D, 1] = weights["b_upd"]
    cpack[0:D, 2] = weights["b_cnf"]
    cpack[0:D, 3] = weights["b_msg"]
    cpack[0:HG, 4] = weights["b_g1"]
    cpack[0:3, 5] = weights["b_g2"]
    cpack[0:3, 6] = 1.0
    m["CPACK"] = cpack
    return m


def kernel(**inputs):
    from concourse.bass_utils import run_bass_kernel_spmd

    cur = np.asarray(inputs["current_state"], np.float32)
    nbr = np.asarray(inputs["neighbor_states"], np.float32)
    conn = np.asarray(inputs["conn_type"], np.int32)
    weights = {k: np.asarray(v, np.float32) for k, v in inputs.items()
               if k not in ("current_state", "neighbor_states", "conn_type")}

    npad = NCORES * NS
    cur_p = np.zeros((npad, D), np.float32)
    cur_p[:N_CELLS] = cur
    nbr_p = np.zeros((npad, K, D), np.float32)
    nbr_p[:N_CELLS] = nbr
    conn_p = np.full((npad, K), 3, np.int32)
    conn_p[:N_CELLS] = conn

    in_maps = []
    for c in range(NCORES):
        sl = slice(c * NS, (c + 1) * NS)
        in_maps.append(_prep_core_inputs(cur_p[sl], nbr_p[sl], conn_p[sl],
                                         weights))
    nc = _get_nc()
    res = run_bass_kernel_spmd(nc, in_maps, list(range(NCORES)))
    out = np.concatenate([res.results[c]["outT"].T for c in range(NCORES)],
                         axis=0)
    return np.ascontiguousarray(out[:N_CELLS]).astype(np.float32)


if __name__ == "__main__":
    pass
